# revision 4
# baseline (speedup 1.0000x reference)
"""Trainium2 Bass kernel for AttentionFixModel (topk_masking).

Computation (per (b,t) row):
  q_proj = queries @ W_in + b_in                       [B,T,D]
  scores = einsum('btd,btnd->btn', q_proj, patch)      [B,T,N]
  attn   = softmax(scores); top-16 hard mask; renorm
  out    = einsum('btn,btnd->btd', attn, patch) @ W_out + b_out

Sharding: data-parallel over batch. B=16 across 8 cores -> 2 batches
(32 rows) per core. Weights replicated (host-packed to f16).

v2 strategy: real HW charges serial LD_WEIGHTS (~1 col/cycle) per
matmul, so the v1 all-PE score pipeline (transpose + 128-col stationary
per 1-col moving matmul) was PE-bound at ~63us.  v2 computes scores for
most rows on DVE via tensor_tensor_reduce(patch_f16 * qrep_f16 ->
accum f32) directly on the native [n-partition, d-free] layout - no
patch transposes at all.  qrep (q_proj broadcast to 128 partitions)
comes from a free-axis stride-0 broadcast DMA.  A few early rows keep
the v1 PE path (transpose + score matmul) to balance engine load while
qrep is not yet ready.  Weighted-sum/projections stay on PE (native
patch stationary).  Patches stream as n=2p+h interleaved cast-DMA
chunks (3072B descriptors).  Group epilogues (top-16 via max8 +
match_replace + max8, exp overlap on ACT, diag-renorm folded into the
weight transpose) are software-pipelined against later rows' scores;
final groups are small to shrink the post-DMA tail.
"""
import os
import sys

for _p in ("/opt/trn_rl_repo", "/root/.axon_site/_ro/trn_rl_repo"):
    if _p not in sys.path and os.path.isdir(_p):
        sys.path.append(_p)

import numpy as np
import concourse.bass as bass
import concourse.bacc as bacc
import concourse.mybir as mybir
from concourse import masks
from concourse.tile import TileContext

F32 = mybir.dt.float32
F16 = mybir.dt.float16
Alu = mybir.AluOpType
Act = mybir.ActivationFunctionType

B, T, N, D = 16, 16, 256, 384
QDIM = 384
TOPK = 16
EPS = 1e-8
NEG = -1e30
NCORES = 8
BT = (B // NCORES) * T          # rows per core = 32
NH = N // 128                   # patch partition-halves (2)
ND = D // 128                   # d-dim 128-tiles (3)
NQ = QDIM // 128                # q-dim 128-tiles (3)
NK = NH * ND                    # patchT chunks per PE row (6)

# patch DMA chunk sizes (rows); PE-path rows; topk/wsum groups
CHUNKS = [2, 2, 4, 4, 4, 4, 4, 2, 2, 2, 1, 1]
PE_NROWS = 10                   # rows 0..PE_NROWS-1 use the v1 PE path
GROUPS = [16, 8, 4, 4]
WARMUP_MMS = 8

W_IN_OFF = 0
W_OUT_OFF = NQ * D              # 1152
WGT_COLS = 2 * NQ * D           # 2304


def build_kernel() -> bass.Bass:
    nc = bacc.Bacc("TRN2", target_bir_lowering=False)

    sm_d = nc.dram_tensor("smalls", [BT + 33, QDIM], F16, kind="ExternalInput")
    wgt_d = nc.dram_tensor("wgt", [128, WGT_COLS], F16, kind="ExternalInput")
    p_d = nc.dram_tensor("patch_features", [BT, N, D], F32, kind="ExternalInput")
    out_d = nc.dram_tensor("out", [BT, QDIM], F32, kind="ExternalOutput")

    # DRAM view of patches: [p=128, bt, h, d] with n = 2p + h so each
    # (p, bt) source run is 2 rows = 3072B contiguous (big descriptors)
    p_view = p_d[:].rearrange("bt (p h) d -> p bt h d", h=NH)

    with TileContext(nc) as tc:
        with (
            tc.tile_pool(name="const", bufs=1) as cpool,
            tc.tile_pool(name="wgt", bufs=1) as wpool,
            tc.tile_pool(name="patch", bufs=1) as ppool,
            tc.tile_pool(name="pT", bufs=3) as spool,
            tc.tile_pool(name="rows", bufs=2) as rpool,
            tc.tile_pool(name="ptT", bufs=2, space="PSUM") as ptpool,
            tc.tile_pool(name="psc", bufs=1, space="PSUM") as scpool,
            tc.tile_pool(name="poc", bufs=1, space="PSUM") as ocpool,
            tc.tile_pool(name="ptr", bufs=2, space="PSUM") as trpool,
            tc.tile_pool(name="pfin", bufs=1, space="PSUM") as finpool,
        ):
            # ---------- small DMAs (HWDGE); W_out is loaded LAST ----------
            smalls = wpool.tile([BT + 33, QDIM], F16, tag="smalls")
            nc.sync.dma_start(smalls[:], sm_d[:])
            wgt = wpool.tile([128, WGT_COLS], F16, tag="wgt")
            nc.sync.dma_start(wgt[:, :W_OUT_OFF], wgt_d[:, :W_OUT_OFF])
            queries = smalls[:BT, :]
            b_in = smalls[32:33, :]
            b_out = smalls[64:65, :]
            w_in = [wgt[:, W_IN_OFF + j * D:W_IN_OFF + (j + 1) * D]
                    for j in range(NQ)]
            w_out = [wgt[:, W_OUT_OFF + j * QDIM:W_OUT_OFF + (j + 1) * QDIM]
                     for j in range(ND)]

            # ---------- patch cast-DMAs (SWDGE, fp32 -> f16) ----------
            ident16 = cpool.tile([128, 128], F16)
            ident32 = cpool.tile([128, 128], F32)
            ones16 = cpool.tile([BT + 33, 128], F16)
            rows = []                     # global row -> (tile, idx)
            cb = 0
            for k, sz in enumerate(CHUNKS):
                pk = ppool.tile([128, sz, NH, D], F16, tag=f"patch{k}",
                                name=f"patch{k}")
                nc.gpsimd.dma_start(pk[:], p_view[:, cb:cb + sz])
                rows += [(pk, i) for i in range(sz)]
                cb += sz
                if k == 0:
                    # constants ride behind the first prep
                    masks.make_identity(nc, ident16[:])
                    masks.make_identity(nc, ident32[:])
                    nc.vector.memset(ones16[:], 1.0)
            # W_out generated after every patch prep: its transfer queues
            # behind all patch chunks, landing just before the final chain
            nc.gpsimd.dma_start(wgt[:, W_OUT_OFF:], wgt_d[:, W_OUT_OFF:])

            # ---------- PE p-state warm-up while DMAs land ----------
            qp_ps = finpool.tile([BT, QDIM], F32, tag="pfin")
            for i in range(WARMUP_MMS):
                nc.tensor.matmul(qp_ps[0:1, :D], ones16[0:1, 0:1],
                                 smalls[0:1, :D], start=True, stop=True)

            # ---------- q_proj = queries @ W_in + b_in (all f16) ----------
            qtr = trpool.tile([128, NQ, BT], F16, tag="tr")
            for j in range(NQ):
                nc.tensor.transpose(qtr[:, j, :],
                                    queries[:, 128 * j:128 * (j + 1)],
                                    ident16[:BT, :BT])
            qT0 = wpool.tile([128, NQ, BT], F16, tag="qT0")
            nc.vector.tensor_copy(qT0[:], qtr[:])
            qproj = wpool.tile([BT, D], F16, tag="qproj")
            for j in range(NQ):
                nc.tensor.matmul(qp_ps[:, :D], qT0[:, j, :], w_in[j],
                                 start=(j == 0), stop=False)
            nc.tensor.matmul(qp_ps[:, :D], ones16[32:33, :BT], b_in,
                             start=False, stop=True)
            nc.scalar.copy(qproj[:], qp_ps[:, :D])

            # transposed q_proj columns for the PE-path rows only
            qT = wpool.tile([128, NQ, PE_NROWS], F16, tag="qT")
            qptr = trpool.tile([128, NQ, PE_NROWS], F16, tag="tr", name="qptr")
            for j in range(NQ):
                nc.tensor.transpose(qptr[:, j, :],
                                    qproj[:PE_NROWS, 128 * j:128 * (j + 1)],
                                    ident16[:PE_NROWS, :PE_NROWS])
            nc.vector.tensor_copy(qT[:], qptr[:])

            # qrep: q_proj rows broadcast to all 128 partitions (DVE rows)
            # via free-axis stride-0 broadcast DMAs on the HWDGE queue
            NDVE = BT - PE_NROWS
            qrep = wpool.tile([128, NDVE, D], F16, tag="qrep")
            for r in range(PE_NROWS, BT):
                nc.sync.dma_start(
                    qrep[:, r - PE_NROWS, :],
                    qproj[r:r + 1, :].unsqueeze(1).broadcast_to((1, 128, D)))

            # ---------- group/row bookkeeping ----------
            groups = []
            row_group = {}
            r0 = 0
            for g, sz in enumerate(GROUPS):
                groups.append((r0, sz))
                for r in range(r0, r0 + sz):
                    row_group[r] = (g, r0, sz)
                r0 += sz

            # per-group score-column tiles [128, nr, NH] f32 in SBUF
            scol = {}
            for g, (r0g, nr) in enumerate(groups):
                scol[g] = rpool.tile([128, 16, NH], F32, tag="scol",
                                     name=f"scol{g}")

            # DVE ttr scratch (product side; same-engine serial reuse)
            ttr_scratch = wpool.tile([128, D], F16, tag="ttrscratch")

            # PE-path score PSUM columns (all PE rows are in group 0)
            scpe = scpool.tile([128, PE_NROWS, NH], F32, tag="pscpe")

            # ---------- per-row score emission ----------
            pend_mms = []                 # PE rows transposed, MMs pending

            def emit_pe_score_mms(r):
                pT = pend_pT.pop(r)
                for h in range(NH):
                    for j in range(ND):
                        nc.tensor.matmul(scpe[:, r, h:h + 1],
                                         pT[:, h * ND + j, :],
                                         qT[:, j, r:r + 1],
                                         start=(j == 0), stop=(j == ND - 1))

            pend_pT = {}

            def emit_pe_row(r, trail=2):
                pc, i = rows[r]
                ptr_ps = ptpool.tile([128, NK, 128], F16, tag="ptT")
                for h in range(NH):
                    for j in range(ND):
                        nc.tensor.transpose(
                            ptr_ps[:, h * ND + j, :],
                            pc[:, i, h, 128 * j:128 * (j + 1)],
                            ident16[:, :])
                pT = spool.tile([128, NK, 128], F16, tag="pT")
                # f32-bitcast halves the element count the copy engine sees
                nc.scalar.copy(pT[:].bitcast(F32), ptr_ps[:].bitcast(F32))
                pend_pT[r] = pT
                pend_mms.append(r)
                while len(pend_mms) > trail:
                    emit_pe_score_mms(pend_mms.pop(0))

            def flush_pe_rows():
                while pend_mms:
                    emit_pe_score_mms(pend_mms.pop(0))

            def emit_dve_row(r):
                g, r0g, nr = row_group[r]
                pc, i = rows[r]
                for h in range(NH):
                    nc.vector.scalar_tensor_tensor(
                        out=ttr_scratch[:],
                        in0=pc[:, i, h, :],
                        scalar=1.0,
                        in1=qrep[:, r - PE_NROWS, :],
                        op0=Alu.mult, op1=Alu.mult,
                        accum_out=scol[g][:, r - r0g, h:h + 1])

            def emit_row(r):
                if r < PE_NROWS:
                    emit_pe_row(r)
                else:
                    emit_dve_row(r)

            # ---------- group epilogue parts ----------
            def epilogue_parts(g):
                r0g, nr = groups[g]
                st = {}

                def part_a():
                    # scores to row-major [nr, 256]; n' = h*128 + p
                    tr = trpool.tile([16, N], F32, tag="tr", name=f"str{g}")
                    for h in range(NH):
                        nc.tensor.transpose(tr[:nr, 128 * h:128 * (h + 1)],
                                            scol[g][:, :nr, h], ident32[:, :])
                    srows = rpool.tile([16, N], F32, tag="srows")
                    nc.scalar.copy(srows[:nr, :], tr[:nr, :])
                    st.update(srows=srows)
                    # top-16 on raw scores: two max8+match_replace rounds
                    m8a = rpool.tile([16, 8], F32, tag="m8a")
                    nc.vector.max(out=m8a[:nr, :], in_=srows[:nr, :])
                    negm = rpool.tile([16, 1], F32, tag="negm")
                    nc.vector.tensor_scalar(out=negm[:nr, :],
                                            in0=m8a[:nr, 0:1], scalar1=-1.0,
                                            scalar2=None, op0=Alu.mult)
                    st.update(m8a=m8a, negm=negm)

                def part_b():
                    # exp (with z accumulation) overlaps the DVE top-16 hunt
                    p_sb = rpool.tile([16, N], F32, tag="p")
                    zden = rpool.tile([16, 1], F32, tag="z")
                    nc.scalar.activation(out=p_sb[:nr, :],
                                         in_=st["srows"][:nr, :],
                                         func=Act.Exp, bias=st["negm"][:nr, :],
                                         scale=1.0, accum_out=zden[:nr, :])
                    w1 = rpool.tile([16, N], F32, tag="w1")
                    nc.vector.match_replace(out=w1[:nr, :],
                                            in_to_replace=st["m8a"][:nr, :],
                                            in_values=st["srows"][:nr, :],
                                            imm_value=NEG)
                    m8b = rpool.tile([16, 8], F32, tag="m8b")
                    nc.vector.max(out=m8b[:nr, :], in_=w1[:nr, :])
                    # pm = p where s >= (16th largest), else 0; tsum = sum(pm)
                    pm = rpool.tile([16, N], F32, tag="pm")
                    tsum = rpool.tile([16, 1], F32, tag="t")
                    nc.vector.scalar_tensor_tensor(
                        out=pm[:nr, :], in0=st["srows"][:nr, :],
                        scalar=m8b[:nr, 7:8], in1=p_sb[:nr, :],
                        op0=Alu.is_ge, op1=Alu.mult,
                        accum_out=tsum[:nr, :])
                    den = rpool.tile([16, 1], F32, tag="den")
                    nc.vector.tensor_scalar(out=den[:nr, :],
                                            in0=zden[:nr, :],
                                            scalar1=EPS, scalar2=tsum[:nr, :],
                                            op0=Alu.mult, op1=Alu.add)
                    winv = rpool.tile([16, 1], F32, tag="winv")
                    nc.vector.reciprocal(out=winv[:nr, :], in_=den[:nr, :])
                    # diag(winv): the weight transposes scale their columns,
                    # so the renormalization rides the transpose for free
                    diagw = rpool.tile([16, 16], F32, tag="diagw")
                    nc.vector.tensor_mul(
                        diagw[:nr, :nr], ident32[:nr, :nr],
                        winv[:nr, 0:1].broadcast_to((nr, nr)))
                    st.update(pm=pm, diagw=diagw)

                def part_c():
                    wtr = trpool.tile([128, NH, 16], F32, tag="tr",
                                      name=f"wtr{g}")
                    for h in range(NH):
                        nc.tensor.matmul(wtr[:, h, :nr],
                                         st["pm"][:nr, 128 * h:128 * (h + 1)],
                                         st["diagw"][:nr, :nr],
                                         start=True, stop=True)
                    wcol = rpool.tile([128, NH, 16], F16, tag="wcol")
                    nc.vector.tensor_copy(wcol[:, :, :nr], wtr[:, :, :nr])
                    oc_ps = ocpool.tile([128, ND, 16], F32, tag="poc")
                    fin_ps = finpool.tile([BT, QDIM], F32, tag="pfin")
                    nc.tensor.matmul(fin_ps[:nr, :], ones16[64:65, :nr], b_out,
                                     start=True, stop=False)
                    st.update(wcol=wcol, oc_ps=oc_ps, fin_ps=fin_ps)
                    for rl in range(nr):
                        pc, i = rows[r0g + rl]
                        for j in range(ND):
                            for h in range(NH):
                                nc.tensor.matmul(
                                    oc_ps[:, j, rl:rl + 1],
                                    pc[:, i, h, 128 * j:128 * (j + 1)],
                                    st["wcol"][:, h, rl:rl + 1],
                                    start=(h == 0), stop=(h == NH - 1))

                def part_d():
                    oc16 = rpool.tile([128, ND, 16], F16, tag="oc16")
                    nc.vector.tensor_copy(oc16[:, :, :nr],
                                          st["oc_ps"][:, :, :nr])
                    fin_ps = st["fin_ps"]
                    for j in range(ND):
                        nc.tensor.matmul(fin_ps[:nr, :], oc16[:, j, :nr],
                                         w_out[j], start=False,
                                         stop=(j == ND - 1))

                def part_e():
                    fin_ps = st["fin_ps"]
                    fin_sb = rpool.tile([16, QDIM], F32, tag="fin")
                    nc.scalar.copy(fin_sb[:nr, :], fin_ps[:nr, :])
                    nc.sync.dma_start(out_d[r0g:r0g + nr, :], fin_sb[:nr, :])

                return [part_a, part_b, part_c, part_d, part_e]

            # ---------- software-pipelined emission ----------
            # group 0: PE rows 0..9 then DVE rows 10..15
            for r in range(PE_NROWS):
                emit_pe_row(r)
            flush_pe_rows()
            for r in range(PE_NROWS, 16):
                emit_dve_row(r)
            # PE-path scores PSUM -> the group-0 SBUF scol tile
            nc.vector.tensor_copy(scol[0][:, :PE_NROWS, :],
                                  scpe[:, :PE_NROWS, :])

            P0 = epilogue_parts(0)
            P1 = epilogue_parts(1)
            P2 = epilogue_parts(2)
            P3 = epilogue_parts(3)

            P0[0]()                      # A0
            for r in range(16, 20):
                emit_dve_row(r)
            P0[1]()                      # B0
            for r in range(20, 24):
                emit_dve_row(r)
            P0[2]()                      # C0
            P0[3]()                      # D0
            P0[4]()                      # E0
            P1[0]()                      # A1
            for r in range(24, 28):
                emit_dve_row(r)
            P1[1]()                      # B1
            P1[2]()                      # C1
            for r in range(28, 32):
                emit_dve_row(r)
            P1[3]()                      # D1
            P1[4]()                      # E1
            for part in P2:
                part()
            for part in P3:
                part()

    if not nc.is_finalized():
        nc.finalize()
    return nc


def make_in_maps(queries, patch, W_in, b_in, W_out, b_out):
    bpc = B // NCORES
    wgt = np.zeros((128, WGT_COLS), np.float16)
    wgt[:, W_IN_OFF:W_IN_OFF + NQ * D] = (
        W_in.reshape(NQ, 128, D).transpose(1, 0, 2).reshape(128, NQ * D))
    wgt[:, W_OUT_OFF:W_OUT_OFF + ND * QDIM] = (
        W_out.reshape(ND, 128, QDIM).transpose(1, 0, 2).reshape(128, ND * QDIM))
    in_maps = []
    for c in range(NCORES):
        smalls = np.zeros((BT + 33, QDIM), np.float16)
        smalls[:BT] = queries[c * bpc:(c + 1) * bpc].reshape(BT, QDIM)
        smalls[32] = b_in[0]
        smalls[64] = b_out[0]
        in_maps.append({
            "smalls": smalls,
            "wgt": wgt,
            "patch_features": np.ascontiguousarray(
                patch[c * bpc:(c + 1) * bpc].reshape(BT, N, D)),
        })
    return in_maps


_NC_CACHE = None


def kernel(**inputs) -> np.ndarray:
    global _NC_CACHE
    from concourse.bass_utils import run_bass_kernel_spmd

    queries = np.ascontiguousarray(inputs["queries"], dtype=np.float32)
    patch = np.ascontiguousarray(inputs["patch_features"], dtype=np.float32)
    W_in = np.ascontiguousarray(inputs["W_in"], dtype=np.float32)
    b_in = np.ascontiguousarray(inputs["b_in"], dtype=np.float32).reshape(1, D)
    b_out = np.ascontiguousarray(inputs["b_out"], dtype=np.float32).reshape(1, QDIM)
    W_out = np.ascontiguousarray(inputs["W_out"], dtype=np.float32)

    if _NC_CACHE is None:
        _NC_CACHE = build_kernel()
    nc = _NC_CACHE

    in_maps = make_in_maps(queries, patch, W_in, b_in, W_out, b_out)
    res = run_bass_kernel_spmd(nc, in_maps, core_ids=list(range(NCORES)))
    bpc = B // NCORES
    outs = [res.results[c]["out"].reshape(bpc, T, QDIM) for c in range(NCORES)]
    return np.concatenate(outs, axis=0)


# revision 41
# speedup vs baseline: 1.1352x; 1.1352x over previous
"""Trainium2 Bass kernel for AttentionFixModel (topk_masking).

Computation (per (b,t) row):
  q_proj = queries @ W_in + b_in                       [B,T,D]
  scores = einsum('btd,btnd->btn', q_proj, patch)      [B,T,N]
  attn   = softmax(scores); top-16 hard mask; renorm
  out    = einsum('btn,btnd->btd', attn, patch) @ W_out + b_out

Sharding: data-parallel over batch. B=16 across 8 cores -> 2 batches
(32 rows) per core. Weights replicated (host-packed to f16).

v5 strategy: real HW charges serial LD_WEIGHTS (~1 col/cycle) per
matmul, so the v1 all-PE score pipeline (patch transpose + 128-col
stationary per 1-col moving matmul, 3 PE passes over every patch
element) was PE-bound at ~63us.  v5 splits score work per row between
two engines, interleaved by arrival order so both track the patch DMA
stream end-to-end:
  - EVEN slots: PE path (transpose + score matmul, as v1);
  - ODD slots: DVE scalar_tensor_tensor (patch_f16 * qrep_f16,
    free-axis accumulate to f32) on the native [n-part, d-free] layout
    - no transpose, no LDW.  qrep (q_proj broadcast to 128 partitions)
    comes from PE matmuls with a stride-0 broadcast identity column as
    stationary (SBUF broadcast DMAs measured ~2.5us each on real HW, so
    DMA broadcast is avoided), copied to SBUF f16 on ACT.
Queries are host-packed so PE-slot queries sit in rows 0..NPE-1 and
DVE-slot queries follow (keeps all on-chip access contiguous).  Slots
28/30 ride the DVE path too, keeping real-PE busy (incl. LDW) under
the ~35us fp32-read DMA floor.
Weighted-sum/projections stay on PE (native patch stationary; its LDW
is irreducible but only ~11us).  W_in AND W_out load up-front on the
sync queue (v1 queued W_out behind the whole patch stream, starving
every group's output projection on real HW).  Patches stream as n=2p+h
interleaved cast-DMA chunks (3072B source descriptors).  Group
epilogues (top-16 via max8 + match_replace + max8, exp on ACT with
denominator accumulation) pipeline against later slots' scores; the
1/den renormalization is applied per-partition on the final rows
(b_out host-replicated), keeping the weighted sum off the denominator
chain.  Per group the score rows are ordered [even slots, odd slots];
the output DMA writes each half through a strided DRAM view.
"""
import os
import sys

for _p in ("/opt/trn_rl_repo", "/root/.axon_site/_ro/trn_rl_repo"):
    if _p not in sys.path and os.path.isdir(_p):
        sys.path.append(_p)

import numpy as np
import concourse.bass as bass
import concourse.bacc as bacc
import concourse.mybir as mybir
from concourse import masks
from concourse.tile import TileContext

F32 = mybir.dt.float32
F16 = mybir.dt.float16
Alu = mybir.AluOpType
Act = mybir.ActivationFunctionType

B, T, N, D = 16, 16, 256, 384
QDIM = 384
TOPK = 16
EPS = 1e-8
NEG = -1e30
NCORES = 8
BT = (B // NCORES) * T          # rows per core = 32
NH = N // 128                   # patch partition-halves (2)
ND = D // 128                   # d-dim 128-tiles (3)
NQ = QDIM // 128                # q-dim 128-tiles (3)
NK = NH * ND                    # patchT chunks per PE row (6)
# PE path: even slots except 28/30 (those go to DVE to keep real-HW PE
# busy, incl. serial LD_WEIGHTS, under the DMA floor); DVE: the rest
DVE_EXTRA = {28, 30}
PE_SLOTS = [s for s in range(32) if s % 2 == 0 and s not in DVE_EXTRA]
DVE_SLOTS = [s for s in range(32) if s % 2 == 1 or s in DVE_EXTRA]
NPE = len(PE_SLOTS)             # 14
NDVE = len(DVE_SLOTS)           # 18
PE_ORD = {s: i for i, s in enumerate(PE_SLOTS)}
DVE_ORD = {s: i for i, s in enumerate(DVE_SLOTS)}

# patch DMA chunk sizes (rows); topk/wsum groups
CHUNKS = [2, 2, 4, 4, 4, 4, 4, 2, 2, 2, 1, 1]
GROUPS = [16, 8, 6, 2]
PSUM_TOPK_GROUPS = {3}          # groups whose topk reads scores from PSUM
WARMUP_MMS = 8

W_IN_OFF = 0
W_OUT_OFF = NQ * D              # 1152
WGT_COLS = 2 * NQ * D           # 2304


def build_kernel() -> bass.Bass:
    nc = bacc.Bacc("TRN2", target_bir_lowering=False)

    sm_d = nc.dram_tensor("smalls", [BT + 48, QDIM], F16, kind="ExternalInput")
    wgt_d = nc.dram_tensor("wgt", [128, WGT_COLS], F16, kind="ExternalInput")
    p_d = nc.dram_tensor("patch_features", [BT, N, D], F32, kind="ExternalInput")
    out_d = nc.dram_tensor("out", [BT, QDIM], F32, kind="ExternalOutput")

    # DRAM view of patches: [p=128, bt, h, d] with n = 2p + h so each
    # (p, bt) source run is 2 rows = 3072B contiguous (big descriptors)
    p_view = p_d[:].rearrange("bt (p h) d -> p bt h d", h=NH)
    # output viewed [slot-pair, parity, q] for the per-parity strided DMAs
    out_v = out_d[:].rearrange("(s two) q -> s two q", two=2)

    with TileContext(nc) as tc:
        with (
            tc.tile_pool(name="const", bufs=1) as cpool,
            tc.tile_pool(name="wgt", bufs=1) as wpool,
            tc.tile_pool(name="patch", bufs=1) as ppool,
            tc.tile_pool(name="pT", bufs=3) as spool,
            tc.tile_pool(name="rows", bufs=2) as rpool,
            tc.tile_pool(name="ptT", bufs=2, space="PSUM") as ptpool,
            tc.tile_pool(name="psc", bufs=1, space="PSUM") as scpool,
            tc.tile_pool(name="poc", bufs=1, space="PSUM") as ocpool,
            tc.tile_pool(name="ptr", bufs=2, space="PSUM") as trpool,
            tc.tile_pool(name="pfin", bufs=2, space="PSUM") as finpool,
        ):
            # ---------- small DMAs (HWDGE): W_in, queries, W_out ----------
            wgt = wpool.tile([128, WGT_COLS], F16, tag="wgt")
            nc.sync.dma_start(wgt[:, :W_OUT_OFF], wgt_d[:, :W_OUT_OFF])
            smalls = wpool.tile([BT + 48, QDIM], F16, tag="smalls")
            nc.sync.dma_start(smalls[:], sm_d[:])
            # W_out up-front too: late (behind the patch stream) it would
            # starve every group's output projection on real HW
            nc.sync.dma_start(wgt[:, W_OUT_OFF:], wgt_d[:, W_OUT_OFF:])
            queries = smalls[:BT, :]       # row k<16: slot 2k; else slot 2(k-16)+1
            b_in = smalls[32:33, :]
            # b_out host-replicated to 16 rows; staged to a partition-0 f32
            # tile (stt requires equal base partitions across SBUF inputs)
            b_out_rep = cpool.tile([16, QDIM], F32)
            nc.scalar.copy(b_out_rep[:], smalls[64:80, :])
            w_in = [wgt[:, W_IN_OFF + j * D:W_IN_OFF + (j + 1) * D]
                    for j in range(NQ)]
            w_out = [wgt[:, W_OUT_OFF + j * QDIM:W_OUT_OFF + (j + 1) * QDIM]
                     for j in range(ND)]

            # ---------- patch cast-DMAs (SWDGE, fp32 -> f16) ----------
            ident16 = cpool.tile([128, 128], F16)
            ident32 = cpool.tile([128, 128], F32)
            ones16 = cpool.tile([BT + 33, 128], F16)
            rows = []                     # slot -> (tile, idx)
            cb = 0
            for k, sz in enumerate(CHUNKS):
                pk = ppool.tile([128, sz, NH, D], F16, tag=f"patch{k}",
                                name=f"patch{k}")
                if k >= len(CHUNKS) - 2 and sz == 1:
                    # tail chunks land per-half so dependents start earlier
                    for h in range(NH):
                        nc.gpsimd.dma_start(pk[:, :, h, :],
                                            p_view[:, cb:cb + sz, h:h + 1, :])
                else:
                    nc.gpsimd.dma_start(pk[:], p_view[:, cb:cb + sz])
                rows += [(pk, i) for i in range(sz)]
                cb += sz
                if k == 0:
                    # constants ride behind the first prep
                    masks.make_identity(nc, ident16[:])
                    masks.make_identity(nc, ident32[:])
                    nc.vector.memset(ones16[:], 1.0)

            # ---------- PE p-state warm-up while DMAs land ----------
            # (depends only on ones16, not on any DMA)
            qp_ps = finpool.tile([BT, QDIM], F32, tag="pfin")
            for i in range(WARMUP_MMS):
                nc.tensor.matmul(qp_ps[0:1, :128], ones16[0:1, 0:1],
                                 ones16[0:1, :], start=True, stop=True)

            # ---------- q_proj = queries @ W_in + b_in (all f16) ----------
            qtr = trpool.tile([128, NQ, BT], F16, tag="tr")
            for j in range(NQ):
                nc.tensor.transpose(qtr[:, j, :],
                                    queries[:, 128 * j:128 * (j + 1)],
                                    ident16[:BT, :BT])
            qT0 = wpool.tile([128, NQ, BT], F16, tag="qT0")
            nc.vector.tensor_copy(qT0[:], qtr[:])
            qproj = wpool.tile([BT, D], F16, tag="qproj")
            for j in range(NQ):
                nc.tensor.matmul(qp_ps[:, :D], qT0[:, j, :], w_in[j],
                                 start=(j == 0), stop=False)
            nc.tensor.matmul(qp_ps[:, :D], ones16[32:33, :BT], b_in,
                             start=False, stop=True)
            nc.vector.tensor_copy(qproj[:], qp_ps[:, :D])

            # qrep: DVE-slot q_proj rows broadcast to all 128 partitions.
            # Broadcast DMAs cost ~2.5us each on real HW, so broadcast on
            # PE instead: ones-column stationary x q_proj row -> PSUM
            # (raw-aliasing the ptT PSUM ring), then copy to SBUF f16
            # (alternating DVE/ACT).  Emitted interleaved with the early
            # slots so nothing head-of-line blocks.
            qrep = wpool.tile([128, NDVE, D], F16, tag="qrep")

            def emit_qrep(k):
                # identity column NPE+k broadcast along 128 output columns
                # selects q_proj row NPE+k into every PSUM partition
                qr = ptpool.tile([128, NK, 128], F16, tag="ptT",
                                 name=f"qr{k}")
                qr32 = qr[:].rearrange("p a b -> p (a b)").bitcast(F32)
                lhsT = ident16[:BT, NPE + k:NPE + k + 1].broadcast_to((BT, 128))
                nc.tensor.matmul(qr32[:, :D], lhsT, qproj[:, :],
                                 start=True, stop=True)
                nc.scalar.copy(qrep[:, k, :], qr32[:, :D])

            # transposed q_proj columns for the PE slots
            qT = wpool.tile([128, NQ, NPE], F16, tag="qT")
            qptr = trpool.tile([128, NQ, NPE], F16, tag="tr", name="qptr")
            for j in range(NQ):
                nc.tensor.transpose(qptr[:, j, :],
                                    qproj[:NPE, 128 * j:128 * (j + 1)],
                                    ident16[:NPE, :NPE])
            nc.vector.tensor_copy(qT[:], qptr[:])

            # ---------- group/slot bookkeeping ----------
            # per group: score-row order = [PE slots asc, DVE slots asc]
            groups = []
            slot_group = {}
            r0 = 0
            for g, sz in enumerate(GROUPS):
                groups.append((r0, sz))
                for s in range(r0, r0 + sz):
                    slot_group[s] = (g, r0, sz)
                r0 += sz
            g_pe = [[s for s in range(r0g, r0g + nr) if s in PE_ORD]
                    for r0g, nr in groups]
            g_order = [g_pe[g] + [s for s in range(r0g, r0g + nr)
                                  if s in DVE_ORD]
                       for g, (r0g, nr) in enumerate(groups)]
            rl_map = {s: rl for g, order in enumerate(g_order)
                      for rl, s in enumerate(order)}

            def rl_of(s):
                return rl_map[s]

            def slot_of(g, rl):
                return g_order[g][rl]

            scol = {}
            for g, (r0g, nr) in enumerate(groups):
                scol[g] = rpool.tile([128, nr, NH], F32, tag="scol",
                                     name=f"scol{g}")

            # DVE stt scratch (product side; same-engine serial reuse)
            stt_scratch = wpool.tile([128, D], F16, tag="sttscratch")

            # PE-path score PSUM columns, indexed by PE ordinal (slot//2)
            scpe = scpool.tile([128, NPE, NH], F32, tag="pscpe")

            # ---------- per-slot score emission ----------
            pend_mms = []                 # PE slots transposed, MMs pending
            pend_pT = {}

            def emit_pe_score_mms(s):
                pT = pend_pT.pop(s)
                for h in range(NH):
                    for j in range(ND):
                        k = PE_ORD[s]
                        nc.tensor.matmul(scpe[:, k, h:h + 1],
                                         pT[:, h * ND + j, :],
                                         qT[:, j, k:k + 1],
                                         start=(j == 0), stop=(j == ND - 1))

            def emit_pe_row(s, trail=2):
                pc, i = rows[s]
                ptr_ps = ptpool.tile([128, NK, 128], F16, tag="ptT")
                for h in range(NH):
                    for j in range(ND):
                        nc.tensor.transpose(
                            ptr_ps[:, h * ND + j, :],
                            pc[:, i, h, 128 * j:128 * (j + 1)],
                            ident16[:, :])
                pT = spool.tile([128, NK, 128], F16, tag="pT")
                # f32-bitcast halves the element count the copy engine sees
                nc.scalar.copy(pT[:].bitcast(F32), ptr_ps[:].bitcast(F32))
                pend_pT[s] = pT
                pend_mms.append(s)
                while len(pend_mms) > trail:
                    emit_pe_score_mms(pend_mms.pop(0))

            def flush_pe_upto(s_max):
                while pend_mms and pend_mms[0] <= s_max:
                    emit_pe_score_mms(pend_mms.pop(0))

            def emit_dve_row(s):
                g, r0g, nr = slot_group[s]
                pc, i = rows[s]
                for h in range(NH):
                    nc.vector.scalar_tensor_tensor(
                        out=stt_scratch[:],
                        in0=pc[:, i, h, :],
                        scalar=1.0,
                        in1=qrep[:, DVE_ORD[s], :],
                        op0=Alu.mult, op1=Alu.mult,
                        accum_out=scol[g][:, rl_of(s), h:h + 1])

            next_qrep = [0]

            def pump_qreps(upto_slot):
                # qreps only need qproj; emit a little ahead of consumption
                while (next_qrep[0] < NDVE
                       and DVE_SLOTS[next_qrep[0]] <= upto_slot):
                    emit_qrep(next_qrep[0])
                    next_qrep[0] += 1

            def emit_row(s):
                if s in PE_ORD:
                    emit_pe_row(s)
                    pump_qreps(s + 3)
                else:
                    pump_qreps(s)       # safety: own qrep must precede stt
                    emit_dve_row(s)

            def emit_scpe_copy(g):
                # PE-path scores PSUM -> this group's SBUF scol columns
                r0g, nr = groups[g]
                npe_g = len(g_pe[g])
                if npe_g == 0:
                    return
                flush_pe_upto(r0g + nr - 1)
                k0 = PE_ORD[g_pe[g][0]]
                nc.vector.tensor_copy(scol[g][:, :npe_g, :],
                                      scpe[:, k0:k0 + npe_g, :])

            # ---------- group epilogue parts ----------
            def epilogue_parts(g):
                r0g, nr = groups[g]
                late_renorm = (g == len(GROUPS) - 1)
                st = {}

                def part_a():
                    # scores to row-major [nr, 256]; n' = h*128 + p
                    tr = trpool.tile([16, N], F32, tag="tr", name=f"str{g}")
                    for h in range(NH):
                        nc.tensor.transpose(tr[:nr, 128 * h:128 * (h + 1)],
                                            scol[g][:, :nr, h], ident32[:, :])
                    if g in PSUM_TOPK_GROUPS:
                        srows = tr          # topk reads PSUM directly
                    else:
                        srows = rpool.tile([16, N], F32, tag="srows")
                        nc.scalar.copy(srows[:nr, :], tr[:nr, :])
                    st.update(srows=srows)
                    # top-16 on raw scores: two max8+match_replace rounds
                    m8a = rpool.tile([16, 8], F32, tag="m8a")
                    nc.vector.max(out=m8a[:nr, :], in_=srows[:nr, :])
                    negm = rpool.tile([16, 1], F32, tag="negm")
                    nc.vector.tensor_scalar(out=negm[:nr, :],
                                            in0=m8a[:nr, 0:1], scalar1=-1.0,
                                            scalar2=None, op0=Alu.mult)
                    st.update(m8a=m8a, negm=negm)

                def part_b():
                    # exp (with z accumulation) overlaps the DVE top-16 hunt
                    p_sb = rpool.tile([16, N], F32, tag="p")
                    zden = rpool.tile([16, 1], F32, tag="z")
                    nc.scalar.activation(out=p_sb[:nr, :],
                                         in_=st["srows"][:nr, :],
                                         func=Act.Exp, bias=st["negm"][:nr, :],
                                         scale=1.0, accum_out=zden[:nr, :])
                    w1 = rpool.tile([16, N], F32, tag="w1")
                    nc.vector.match_replace(out=w1[:nr, :],
                                            in_to_replace=st["m8a"][:nr, :],
                                            in_values=st["srows"][:nr, :],
                                            imm_value=NEG)
                    m8b = rpool.tile([16, 8], F32, tag="m8b")
                    nc.vector.max(out=m8b[:nr, :], in_=w1[:nr, :])
                    # pm = p where s >= (16th largest), else 0; tsum = sum(pm)
                    pm = rpool.tile([16, N], F32, tag="pm")
                    tsum = rpool.tile([16, 1], F32, tag="t")
                    nc.vector.scalar_tensor_tensor(
                        out=pm[:nr, :], in0=st["srows"][:nr, :],
                        scalar=m8b[:nr, 7:8], in1=p_sb[:nr, :],
                        op0=Alu.is_ge, op1=Alu.mult,
                        accum_out=tsum[:nr, :])
                    # denominator. Tail group: 1/den is applied at part_e on
                    # the final rows, keeping the weighted sum off this
                    # serial chain.  Other groups: classic diag-fold (winv
                    # scales the weight transpose) so part_e is an ACT copy.
                    den = rpool.tile([16, 1], F32, tag="den")
                    nc.vector.tensor_scalar(out=den[:nr, :],
                                            in0=zden[:nr, :],
                                            scalar1=EPS, scalar2=tsum[:nr, :],
                                            op0=Alu.mult, op1=Alu.add)
                    winv = rpool.tile([16, 1], F32, tag="winv")
                    nc.vector.reciprocal(out=winv[:nr, :], in_=den[:nr, :])
                    st.update(pm=pm, winv=winv)
                    if not late_renorm:
                        diagw = rpool.tile([16, 16], F32, tag="diagw")
                        nc.vector.tensor_mul(
                            diagw[:nr, :nr], ident32[:nr, :nr],
                            winv[:nr, 0:1].broadcast_to((nr, nr)))
                        st.update(diagw=diagw)

                def part_c():
                    wtr = trpool.tile([128, NH, 16], F32, tag="tr",
                                      name=f"wtr{g}")
                    for h in range(NH):
                        if late_renorm:
                            nc.tensor.transpose(
                                wtr[:, h, :nr],
                                st["pm"][:nr, 128 * h:128 * (h + 1)],
                                ident32[:nr, :nr])
                        else:
                            nc.tensor.matmul(
                                wtr[:, h, :nr],
                                st["pm"][:nr, 128 * h:128 * (h + 1)],
                                st["diagw"][:nr, :nr],
                                start=True, stop=True)
                    wcol = rpool.tile([128, NH, 16], F16, tag="wcol")
                    nc.vector.tensor_copy(wcol[:, :, :nr], wtr[:, :, :nr])
                    oc_ps = ocpool.tile([128, ND, 16], F32, tag="poc")
                    fin_ps = finpool.tile([BT, QDIM], F32, tag="pfin")
                    if not late_renorm:
                        nc.tensor.matmul(fin_ps[:nr, :], ones16[64:65, :nr],
                                         smalls[64:65, :],
                                         start=True, stop=False)
                    st.update(wcol=wcol, oc_ps=oc_ps, fin_ps=fin_ps)
                    for rl in range(nr):
                        pc, i = rows[slot_of(g, rl)]
                        for j in range(ND):
                            for h in range(NH):
                                nc.tensor.matmul(
                                    oc_ps[:, j, rl:rl + 1],
                                    pc[:, i, h, 128 * j:128 * (j + 1)],
                                    st["wcol"][:, h, rl:rl + 1],
                                    start=(h == 0), stop=(h == NH - 1))

                def part_d():
                    oc16 = rpool.tile([128, ND, 16], F16, tag="oc16")
                    nc.scalar.copy(oc16[:, :, :nr], st["oc_ps"][:, :, :nr])
                    fin_ps = st["fin_ps"]
                    for j in range(ND):
                        nc.tensor.matmul(fin_ps[:nr, :], oc16[:, j, :nr],
                                         w_out[j], start=(late_renorm and j == 0),
                                         stop=(j == ND - 1))

                def part_e():
                    fin_ps = st["fin_ps"]
                    fin_sb = rpool.tile([16, QDIM], F32, tag="fin")
                    if late_renorm:
                        # fin = fin_ps * (1/den) + b_out  (renorm folded here)
                        nc.vector.scalar_tensor_tensor(
                            out=fin_sb[:nr, :], in0=fin_ps[:nr, :],
                            scalar=st["winv"][:nr, 0:1],
                            in1=b_out_rep[:nr, :],
                            op0=Alu.mult, op1=Alu.add)
                    else:
                        nc.scalar.copy(fin_sb[:nr, :], fin_ps[:nr, :])
                    # write maximal runs of fin rows whose slots are
                    # consecutive (stride 1) or alternating (stride 2)
                    order = g_order[g]
                    rl = 0
                    while rl < nr:
                        run = 1
                        if rl + 1 < nr:
                            step = order[rl + 1] - order[rl]
                            if step in (1, 2):
                                while (rl + run < nr and
                                       order[rl + run] - order[rl + run - 1]
                                       == step):
                                    run += 1
                            else:
                                step = 1
                        else:
                            step = 1
                        s0 = order[rl]
                        if step == 1 or run == 1:
                            nc.sync.dma_start(out_d[s0:s0 + run, :],
                                              fin_sb[rl:rl + run, :])
                        else:
                            nc.sync.dma_start(
                                out_v[s0 // 2:s0 // 2 + run, s0 % 2, :],
                                fin_sb[rl:rl + run, :])
                        rl += run

                return [part_a, part_b, part_c, part_d, part_e]

            # ---------- software-pipelined emission ----------
            P = [epilogue_parts(g) for g in range(len(GROUPS))]

            for s in range(0, 16):
                emit_row(s)
            emit_row(16)
            emit_row(17)
            emit_scpe_copy(0)
            P[0][0]()                    # A0 (slots 0-15)
            emit_row(18)
            emit_row(19)
            P[0][1]()                    # B0
            emit_row(20)
            emit_row(21)
            P[0][2]()                    # C0
            emit_row(22)
            emit_row(23)
            P[0][3]()                    # D0
            P[0][4]()                    # E0
            emit_scpe_copy(1)
            P[1][0]()                    # A1 (slots 16-23)
            emit_row(24)
            emit_row(25)
            P[1][1]()                    # B1
            emit_row(26)
            emit_row(27)
            P[1][2]()                    # C1
            emit_row(28)
            emit_row(29)
            P[1][3]()                    # D1
            P[1][4]()                    # E1
            emit_scpe_copy(2)
            P[2][0]()                    # A2 (slots 24-29)
            emit_row(30)
            emit_row(31)
            P[2][1]()                    # B2
            P[2][2]()                    # C2
            emit_scpe_copy(3)
            P[3][0]()                    # A3 (slots 30-31)
            P[3][1]()                    # B3
            P[2][3]()                    # D2
            P[2][4]()                    # E2
            P[3][2]()                    # C3
            P[3][3]()                    # D3
            P[3][4]()                    # E3

    if not nc.is_finalized():
        nc.finalize()
    return nc


def make_in_maps(queries, patch, W_in, b_in, W_out, b_out):
    bpc = B // NCORES
    wgt = np.zeros((128, WGT_COLS), np.float16)
    wgt[:, W_IN_OFF:W_IN_OFF + NQ * D] = (
        W_in.reshape(NQ, 128, D).transpose(1, 0, 2).reshape(128, NQ * D))
    wgt[:, W_OUT_OFF:W_OUT_OFF + ND * QDIM] = (
        W_out.reshape(ND, 128, QDIM).transpose(1, 0, 2).reshape(128, ND * QDIM))
    in_maps = []
    for c in range(NCORES):
        q_core = queries[c * bpc:(c + 1) * bpc].reshape(BT, QDIM)
        smalls = np.zeros((BT + 48, QDIM), np.float16)
        smalls[:NPE] = q_core[PE_SLOTS]
        smalls[NPE:BT] = q_core[DVE_SLOTS]
        smalls[32] = b_in[0]
        smalls[64:80] = b_out[0]
        in_maps.append({
            "smalls": smalls,
            "wgt": wgt,
            "patch_features": np.ascontiguousarray(
                patch[c * bpc:(c + 1) * bpc].reshape(BT, N, D)),
        })
    return in_maps


_NC_CACHE = None


def kernel(**inputs) -> np.ndarray:
    global _NC_CACHE
    from concourse.bass_utils import run_bass_kernel_spmd

    queries = np.ascontiguousarray(inputs["queries"], dtype=np.float32)
    patch = np.ascontiguousarray(inputs["patch_features"], dtype=np.float32)
    W_in = np.ascontiguousarray(inputs["W_in"], dtype=np.float32)
    b_in = np.ascontiguousarray(inputs["b_in"], dtype=np.float32).reshape(1, D)
    b_out = np.ascontiguousarray(inputs["b_out"], dtype=np.float32).reshape(1, QDIM)
    W_out = np.ascontiguousarray(inputs["W_out"], dtype=np.float32)

    if _NC_CACHE is None:
        _NC_CACHE = build_kernel()
    nc = _NC_CACHE

    in_maps = make_in_maps(queries, patch, W_in, b_in, W_out, b_out)
    res = run_bass_kernel_spmd(nc, in_maps, core_ids=list(range(NCORES)))
    bpc = B // NCORES
    outs = [res.results[c]["out"].reshape(bpc, T, QDIM) for c in range(NCORES)]
    return np.concatenate(outs, axis=0)
